# revision 1
# baseline (speedup 1.0000x reference)
"""Trainium2 Bass kernel for ArtistAttentionProcessor (B=6,S=2048,C=320,H=8).

Sharding: tensor-parallel over heads; core h owns head h end-to-end through
attention, then an AllToAll reshards to sequence-slices for the output
projection. All matmul operands are bf16 (PE at 1 cycle/row); PSUM stays f32.

Per core, batches processed in pairs (even at partition base 0, odd at 64):
  - qT/kT/vT = W_h @ hs^T in [40, S] channel-on-partition layout; the
    projections write q+k into one PSUM (M=128 with zero-padded gap rows)
    and v for both batches via column-tiled concurrent matmuls
  - AdaIN over the sequence axis: bn_stats/bn_aggr free-axis stats; the
    scale a = sqrt(var1'/var2') is computed with a DVE-only Newton rsqrt
    so the Scalar engine runs nothing but Exp (single ACT table set)
  - QK^T as scoresT = k @ q^T with K=40 contraction, row-packed across the
    batch pair into one [128, 1024] two-bank PSUM tile; a single Exp per
    k-chunk covers both batches (halves ACT op count)
  - PV accumulates attnT via lhsT = v-chunks [128, 65] with a ones column
    at 64, so the softmax denominator Z lands on an aligned PSUM row
  - normalize via 1/Z broadcast (PE outer-product; kept off gpsimd so the
    collective queue never stalls)
  - two grouped AllToAlls (batches 0-3 after pair 1, 4-5 after pair 2)
    reshard heads -> sequence slices; each core computes its S/8 rows of
    out = attn @ Wo^T via per-head K=40 accumulating matmuls

All host<->device tensors move as few large DMAs (one per batch for the
hs^T load, the A2A send staging, and the gathered recv) — each InstDMACopy
spreads across all 16 SDMA engines but DMAs are FIFO per HW-DGE ring with
~1-2 us fixed cost, so instruction count matters more than transfer shape.

Host side: pre-transposes hs and weights into PE layouts (bf16), runs the
SPMD NEFF on 8 cores via run_bass_kernel_spmd, reassembles [6,2048,320] f32
and adds bo. Measured device time ~0.35-0.52 ms depending on fleet window
(vs 1.01 ms first correct version); rel err vs the f32 reference ~8e-3.
"""

import os
import sys

sys.path.insert(0, "/opt/trn_rl_repo")

import numpy as np

import concourse.bass as bass
import concourse.tile as tile
from concourse import bacc, mybir
from concourse.masks import make_identity

B, S, C, H, D = 6, 2048, 320, 8, 40
NCORES = 8
SSH = S // NCORES  # 256, sequence slice per core after AllToAll
EPS = 1e-5
SCALE = 1.0 / float(np.sqrt(D))
F32 = mybir.dt.float32
# dtype used for matmul operands (flip to float32r / bfloat16 for speed)
MM_DT = mybir.dt.bfloat16


def fr(ap):
    return ap
KCH = [(0, 128), (128, 128), (256, 64)]  # contraction chunks of C=320


def build_nc(reps=1, collectives=True):
    nc = bacc.Bacc("TRN2", target_bir_lowering=False, debug=False,
                   num_devices=NCORES)

    hst = nc.dram_tensor("hst", [B, 384, S], MM_DT, kind="ExternalInput").ap()
    wqk = nc.dram_tensor("wqk", [C, 128], MM_DT, kind="ExternalInput").ap()
    wv = nc.dram_tensor("wv", [C, D], MM_DT, kind="ExternalInput").ap()
    wot = nc.dram_tensor("wot", [H, D, C], MM_DT, kind="ExternalInput").ap()
    out = nc.dram_tensor("out", [B, SSH, C], MM_DT, kind="ExternalOutput").ap()

    from contextlib import ExitStack

    with tile.TileContext(nc) as tc:
        with ExitStack() as ctx:
            singles = ctx.enter_context(tc.tile_pool(name="singles", bufs=1))
            hstp = ctx.enter_context(tc.tile_pool(name="hstp", bufs=4))
            qkp = ctx.enter_context(tc.tile_pool(name="qkp", bufs=2))
            k0p = ctx.enter_context(tc.tile_pool(name="k0p", bufs=2))
            vtp = ctx.enter_context(tc.tile_pool(name="vtp", bufs=2))
            vp = ctx.enter_context(tc.tile_pool(name="vp", bufs=2))
            tmpp = ctx.enter_context(tc.tile_pool(name="tmpp", bufs=2))
            probsp = ctx.enter_context(tc.tile_pool(name="probsp", bufs=6))
            attnp = ctx.enter_context(tc.tile_pool(name="attnp", bufs=2))
            recvp = ctx.enter_context(tc.tile_pool(name="recvp", bufs=2))
            outp = ctx.enter_context(tc.tile_pool(name="outp", bufs=2))
            statp = ctx.enter_context(tc.tile_pool(name="statp", bufs=8))
            rzp = ctx.enter_context(tc.tile_pool(name="rzp", bufs=2))
            bcp = ctx.enter_context(tc.tile_pool(name="bcp", bufs=2))
            ps_sc = ctx.enter_context(
                tc.tile_pool(name="ps_sc", bufs=2, space="PSUM"))
            ps_at = ctx.enter_context(
                tc.tile_pool(name="ps_at", bufs=2, space="PSUM"))
            ps_misc = ctx.enter_context(
                tc.tile_pool(name="ps_misc", bufs=2, space="PSUM"))
            dramp = ctx.enter_context(
                tc.tile_pool(name="dramp", bufs=1, space="DRAM"))

            # ---- constants ----
            ident = singles.tile([128, 128], MM_DT)
            make_identity(nc, ident)
            ones40 = singles.tile([1, D], MM_DT)
            nc.vector.memset(ones40, 1.0)
            eps_t = singles.tile([128, 1], F32)
            nc.vector.memset(eps_t, EPS)

            wqk_sb = singles.tile([128, 3, 128], MM_DT)
            wv_sb = singles.tile([128, 3, D], MM_DT)
            wot_sb = singles.tile([D, H, C], MM_DT)
            for ck, (k0, kn) in enumerate(KCH):
                nc.sync.dma_start(wqk_sb[0:kn, ck, :], wqk[k0:k0 + kn, :])
                nc.sync.dma_start(wv_sb[0:kn, ck, :], wv[k0:k0 + kn, :])
            for h in range(H):
                nc.sync.dma_start(wot_sb[:, h, :], wot[h])

            # style stats saved from batch 1 (for 2) and 4 (for 5),
            # mirrored at partition bases 0 and 64 for both pair halves
            style = {n: singles.tile([128, 2], F32, tag=f"style_{n}",
                                     name=f"style_{n}")
                     for n in ("q", "k", "v")}

            def seq_stats(x, base):
                """mean and var' = var*S/(S-1) + eps over the free axis of
                x [D, S] (no ACT usage; keeps ACT exp-only)."""
                o = slice(base, base + D)
                st = statp.tile([128, 4, 6], F32, tag="bnst", name="st")
                for g in range(4):
                    nc.vector.bn_stats(st[o, g, :], x[:, g * 512:(g + 1) * 512])
                mv = statp.tile([128, 2], F32, tag="mv", name="mv")
                nc.vector.bn_aggr(mv[o, :], st[o])
                vp_ = statp.tile([128, 1], F32, tag="vp_", name="vp_")
                nc.vector.tensor_scalar(
                    out=vp_[o], in0=mv[o, 1:2],
                    scalar1=float(S) / float(S - 1), scalar2=EPS,
                    op0=mybir.AluOpType.mult, op1=mybir.AluOpType.add)
                return mv, vp_

            def adain(x, name, b, base):
                """AdaIN on x [D, S] at partition base 0/64: save style
                stats at b in {1,4}; restyle at b in {2,5}.
                a = sqrt(vs'/v') via r*rsqrt(r), rsqrt by 3 Newton steps
                from y0=1 (exact enough for var ratios near 1)."""
                if b in (0, 3):
                    return
                o = slice(base, base + D)
                mv, vp_ = seq_stats(x, base)
                if b in (1, 4):
                    for dst in (slice(0, D), slice(64, 64 + D)):
                        nc.vector.tensor_copy(style[name][dst, 0:1],
                                              mv[o, 0:1])
                        nc.vector.tensor_copy(style[name][dst, 1:2], vp_[o])
                else:
                    r = statp.tile([128, 1], F32, tag="r", name="r")
                    nc.vector.reciprocal(r[o], vp_[o])
                    nc.vector.tensor_mul(r[o], style[name][o, 1:2], r[o])
                    y = statp.tile([128, 1], F32, tag="y", name="y")
                    t = statp.tile([128, 1], F32, tag="t", name="t")
                    # y = (3 - r)/2  (first Newton step from y0=1)
                    nc.vector.tensor_scalar(
                        out=y[o], in0=r[o], scalar1=-0.5, scalar2=1.5,
                        op0=mybir.AluOpType.mult, op1=mybir.AluOpType.add)
                    for _ in range(2):
                        nc.vector.tensor_mul(t[o], y[o], y[o])
                        nc.vector.tensor_mul(t[o], t[o], r[o])
                        nc.vector.tensor_scalar(
                            out=t[o], in0=t[o], scalar1=-0.5, scalar2=1.5,
                            op0=mybir.AluOpType.mult, op1=mybir.AluOpType.add)
                        nc.vector.tensor_mul(y[o], y[o], t[o])
                    a = statp.tile([128, 1], F32, tag="a", name="a")
                    nc.vector.tensor_mul(a[o], r[o], y[o])
                    am = statp.tile([128, 1], F32, tag="am", name="am")
                    nc.vector.tensor_mul(am[o], mv[o, 0:1], a[o])
                    bvec = statp.tile([128, 1], F32, tag="bvec", name="bvec")
                    nc.vector.tensor_sub(bvec[o], style[name][o, 0:1], am[o])
                    nc.vector.tensor_scalar(
                        out=x, in0=x, scalar1=a[o], scalar2=bvec[o],
                        op0=mybir.AluOpType.mult, op1=mybir.AluOpType.add)

            for _rep in range(reps):
              sends, recvs = [], []
              for g, nb in ((0, 4), (1, 2)):
                  sends.append(dramp.tile([NCORES, nb, D, SSH], MM_DT,
                                          tag=f"sendg{g}", name=f"sendg{g}"))
                  recvs.append(dramp.tile([NCORES, nb, D, SSH], MM_DT,
                                          tag=f"recvg{g}", name=f"recvg{g}"))
              for p in range(B // 2):
                be, bo = 2 * p, 2 * p + 1
                # ---- load hs^T for both batches of the pair ----
                hst_e = hstp.tile([128, 3, S], MM_DT, tag="hst", name="hst_e")
                hst_o = hstp.tile([128, 3, S], MM_DT, tag="hst", name="hst_o")
                for hs_sb, bb in ((hst_e, be), (hst_o, bo)):
                    nc.sync.dma_start(
                        hs_sb[:, :, :],
                        hst[bb].rearrange("(ck p) s -> p ck s", p=128))

                # pair layout: even batch at partition base 0, odd at base 64
                qT2 = qkp.tile([128, S], MM_DT)
                kT2 = k0p.tile([128, S], MM_DT)
                vT2 = vtp.tile([128, S], MM_DT)
                tmp = tmpp.tile([128, S], MM_DT)
                for qc in range(4):
                    ns = slice(qc * 512, (qc + 1) * 512)
                    pqk = ps_misc.tile([128, 512], F32, tag="misc", name="pqk")
                    for ck, (k0, kn) in enumerate(KCH):
                        nc.tensor.matmul(
                            pqk, lhsT=fr(wqk_sb[0:kn, ck, :]),
                            rhs=fr(hst_e[0:kn, ck, ns]),
                            start=(ck == 0), stop=(ck == 2))
                    nc.vector.tensor_copy(qT2[0:D, ns], pqk[0:D, :])
                    nc.vector.tensor_copy(tmp[64:64 + D, ns], pqk[64:64 + D, :])
                    pqk2 = ps_misc.tile([128, 512], F32, tag="misc", name="pqk2")
                    for ck, (k0, kn) in enumerate(KCH):
                        nc.tensor.matmul(
                            pqk2, lhsT=fr(wqk_sb[0:kn, ck, :]),
                            rhs=fr(hst_o[0:kn, ck, ns]),
                            start=(ck == 0), stop=(ck == 2))
                    nc.vector.tensor_copy(kT2[64:64 + D, ns], pqk2[64:64 + D, :])
                    nc.vector.tensor_copy(tmp[0:D, ns], pqk2[0:D, :])
                    # v pair, col-packed: v_e -> psum rows 0:40, v_o -> 64:104
                    pv = ps_misc.tile([128, 512], F32, tag="misc", name="pv")
                    for ck, (k0, kn) in enumerate(KCH):
                        nc.tensor.matmul(
                            pv[0:D, :], lhsT=fr(wv_sb[0:kn, ck, :]),
                            rhs=fr(hst_e[0:kn, ck, ns]),
                            start=(ck == 0), stop=(ck == 2))
                    for ck, (k0, kn) in enumerate(KCH):
                        nc.tensor.matmul(
                            pv[64:64 + D, :], lhsT=fr(wv_sb[0:kn, ck, :]),
                            rhs=fr(hst_o[0:kn, ck, ns]),
                            start=(ck == 0), stop=(ck == 2))
                    nc.vector.tensor_copy(vT2[0:D, ns], pv[0:D, :])
                    nc.vector.tensor_copy(vT2[64:64 + D, ns], pv[64:64 + D, :])
                # shifts: k_e staged at tmp[64:104] -> kT2 base 0;
                # q_o staged at tmp[0:40] -> qT2 base 64
                nc.sync.dma_start(kT2[0:D, :], tmp[64:64 + D, :])
                nc.sync.dma_start(qT2[64:64 + D, :], tmp[0:D, :])

                # ---- AdaIN (even at base 0, odd at base 64) ----
                adain(qT2[0:D, :], "q", be, 0)
                adain(kT2[0:D, :], "k", be, 0)
                adain(vT2[0:D, :], "v", be, 0)
                adain(qT2[64:64 + D, :], "q", bo, 64)
                adain(kT2[64:64 + D, :], "k", bo, 64)
                adain(vT2[64:64 + D, :], "v", bo, 64)

                # ---- v transposes (row-packed pair) ----
                v_sbs = {}
                for half, base in ((0, 0), (1, 64)):
                    v_sb = vp.tile([128, 16, 65], MM_DT, tag=f"v_sb{half}",
                                   name=f"v_sb{half}")
                    nc.vector.memset(v_sb[:, :, D:64], 0.0)
                    nc.vector.memset(v_sb[:, :, 64:65], 1.0)
                    v_sbs[half] = v_sb
                for sc in range(16):
                    for half, base in ((0, 0), (1, 64)):
                        tp = ps_misc.tile([128, 512], MM_DT, tag="misc",
                                          name="tp")
                        nc.tensor.transpose(
                            fr(tp[:, 0:D]),
                            fr(vT2[base:base + D, sc * 128:(sc + 1) * 128]),
                            fr(ident[base:base + D, base:base + D]))
                        nc.vector.tensor_copy(v_sbs[half][:, sc, 0:D],
                                              tp[:, 0:D])

                # ---- attention (QK^T row-packed across the pair) ----
                at_e = attnp.tile([D, S], MM_DT, tag="at_e", name="at_e")
                at_o = attnp.tile([D, S], MM_DT, tag="at_o", name="at_o")
                for qc in range(4):
                    qs = slice(qc * 512, (qc + 1) * 512)
                    pat_e = ps_at.tile([65, 512], F32, tag="attn", name="pat_e")
                    pat_o = ps_at.tile([65, 512], F32, tag="attn", name="pat_o")
                    for kc in range(16):
                        ks = slice(kc * 128, (kc + 1) * 128)
                        psc = ps_sc.tile([128, 1024], F32, tag="scores",
                                         name="psc")
                        nc.tensor.matmul(psc[:, 0:512],
                                         lhsT=fr(kT2[0:D, ks]),
                                         rhs=fr(qT2[0:D, qs]),
                                         start=True, stop=True)
                        nc.tensor.matmul(psc[:, 512:1024],
                                         lhsT=fr(kT2[64:64 + D, ks]),
                                         rhs=fr(qT2[64:64 + D, qs]),
                                         start=True, stop=True)
                        pb = probsp.tile([128, 1024], MM_DT, name="pb")
                        nc.scalar.activation(
                            pb, psc, mybir.ActivationFunctionType.Exp,
                            scale=SCALE)
                        nc.tensor.matmul(pat_e, lhsT=fr(v_sbs[0][:, kc, :]),
                                         rhs=fr(pb[:, 0:512]),
                                         start=(kc == 0), stop=(kc == 15))
                        nc.tensor.matmul(pat_o, lhsT=fr(v_sbs[1][:, kc, :]),
                                         rhs=fr(pb[:, 512:1024]),
                                         start=(kc == 0), stop=(kc == 15))
                    for pat, at in ((pat_e, at_e), (pat_o, at_o)):
                        rz = rzp.tile([1, 512], MM_DT, name="rz")
                        with nc.allow_low_precision(reason="1/Z in bf16"):
                            nc.vector.reciprocal(rz, pat[64:65, :])
                        pbc = ps_misc.tile([128, 512], F32, tag="misc",
                                           name="pbc")
                        nc.tensor.matmul(pbc[0:D, :], lhsT=ones40, rhs=rz,
                                         start=True, stop=True)
                        bc = bcp.tile([D, 512], F32, name="bc")
                        nc.vector.tensor_copy(bc, pbc[0:D, :])
                        nc.vector.tensor_mul(at[:, qs], pat[0:D, :], bc)

                # ---- stage sends into the grouped AllToAll buffer ----
                for b, attnT in ((be, at_e), (bo, at_o)):
                    g, bl = (0, b) if b < 4 else (1, b - 4)
                    nc.sync.dma_start(
                        sends[g][:, bl, :, :].rearrange("j d s -> d j s"),
                        attnT[:, :].rearrange("d (j s) -> d j s", j=NCORES))

                # ---- grouped AllToAll + output projections ----
                def outproj(b, recv_g, bl):
                    ar = recvp.tile([D, H, SSH], MM_DT, name="ar")
                    nc.sync.dma_start(
                        ar[:, :, :],
                        recv_g[:, bl, :, :].rearrange("h d s -> d h s"))
                    ob = outp.tile([128, 2, C], MM_DT, name="ob")
                    for m in range(2):
                        po = ps_misc.tile([128, 512], F32, tag="misc",
                                          name="po")
                        for h in range(H):
                            nc.tensor.matmul(
                                po[:, 0:C],
                                lhsT=fr(ar[:, h, m * 128:(m + 1) * 128]),
                                rhs=fr(wot_sb[:, h, :]),
                                start=(h == 0), stop=(h == H - 1))
                        nc.vector.tensor_copy(ob[:, m, :], po[:, 0:C])
                    nc.sync.dma_start(
                        out[b].rearrange("(m p) f -> p m f", p=128), ob)

                if p == 1:
                    if collectives:
                        nc.gpsimd.collective_compute(
                            "AllToAll", mybir.AluOpType.bypass,
                            replica_groups=[list(range(NCORES))],
                            ins=[sends[0][:, :, :, :].opt()],
                            outs=[recvs[0][:, :, :, :].opt()])
                    for b in range(4):
                        outproj(b, recvs[0], b)
                elif p == 2:
                    if collectives:
                        nc.gpsimd.collective_compute(
                            "AllToAll", mybir.AluOpType.bypass,
                            replica_groups=[list(range(NCORES))],
                            ins=[sends[1][:, :, :, :].opt()],
                            outs=[recvs[1][:, :, :, :].opt()])
                    for b in range(4, 6):
                        outproj(b, recvs[1], b - 4)

    nc.compile()
    return nc


_NC_CACHE = {}


def _get_nc(reps=1):
    key = f"nc{reps}"
    if key not in _NC_CACHE:
        _NC_CACHE[key] = build_nc(reps)
    return _NC_CACHE[key]


import ml_dtypes


def _np_mm(x):
    return np.ascontiguousarray(np.asarray(x).astype(ml_dtypes.bfloat16))


def make_in_maps(hidden_states, Wq, Wk, Wv, Wo):
    hsT = np.zeros((B, 384, S), np.float32)
    hsT[:, 0:C, :] = np.transpose(np.asarray(hidden_states, np.float32),
                                  (0, 2, 1))
    hsT = _np_mm(hsT)  # [B, 384, S] bf16, rows 320:384 zero
    woT = _np_mm(np.ascontiguousarray(Wo.T).reshape(H, D, C))
    in_maps = []
    for h in range(NCORES):
        sl = slice(h * D, (h + 1) * D)
        stack = np.zeros((128, C), np.float32)
        stack[0:D] = Wq[sl]
        stack[64:64 + D] = Wk[sl]
        in_maps.append({
            "hst": hsT,
            "wqk": _np_mm(stack.T),
            "wv": _np_mm(Wv[sl].T),
            "wot": woT,
        })
    return in_maps


def assemble(results, bo):
    full = np.empty((B, S, C), np.float32)
    for i in range(NCORES):
        full[:, i * SSH:(i + 1) * SSH, :] = results[i]["out"]
    full += np.asarray(bo, np.float32)[None, None, :]
    return full


def run_hw(inputs, trace=False):
    from concourse.bass_utils import run_bass_kernel_spmd
    nc = _get_nc()
    in_maps = make_in_maps(inputs["hidden_states"], inputs["Wq"],
                           inputs["Wk"], inputs["Wv"], inputs["Wo"])
    res = run_bass_kernel_spmd(nc, in_maps, core_ids=list(range(NCORES)),
                               trace=trace)
    return assemble(res.results, inputs["bo"]), res


def kernel(**inputs):
    out, _ = run_hw(inputs, trace=False)
    return out



# revision 35
# speedup vs baseline: 5.0011x; 5.0011x over previous
"""Trainium2 Bass kernel for ArtistAttentionProcessor (B=6,S=2048,C=320,H=8).

Sharding: tensor-parallel over heads; core h owns head h end-to-end through
attention, then an AllToAll reshards to sequence-slices for the output
projection. All matmul operands are bf16 (PE at 1 cycle/row); PSUM stays f32.

Per core, batches processed in pairs (even at partition base 0, odd at 64):
  - qT/kT/vT = W_h @ hs^T in [40, S] channel-on-partition layout; hs^T
    arrives in four 512-column chunk DMAs so projections start early
  - AdaIN over the sequence axis: bn_stats/bn_aggr free-axis stats; the
    scale a = sqrt(var1'/var2') is computed with a DVE-only Newton rsqrt
    so the Scalar engine runs nothing but Exp (single ACT table set)
  - QK^T as scoresT = k @ q^T row-packed across the batch pair into one
    [128, 1024] two-bank PSUM tile; a single Exp per k-chunk covers both
    batches (halves ACT op count)
  - PV accumulates attnT via lhsT = v-chunks [128, 65] with a ones column
    at 64, so the softmax denominator Z lands on an aligned PSUM row; the
    PV matmuls of chunk kc are emitted AFTER the QK+exp of chunk kc+1
    (software pipeline) so the in-order PE queue never parks on the
    ACT-gated PV ahead of the next chunk's QK
  - normalize via 1/Z gpsimd partition_broadcast (keeps PE and PSUM out
    of the normalize; Pool is otherwise idle)
  - pair p+1's QKV projection chunks are emitted inside pair p's
    (ACT-bound) attention qc loop via post_qc callbacks, and group-0
    output projections inside pair 2's, so PE never drains between the
    three attention phases; two grouped AllToAlls (batches 0-3 after
    pair 1, 4-5 after pair 2) reshard heads -> sequence slices
  - out = attn @ Wo^T contracts two heads per matmul (K=80 recv packing,
    4 matmuls per 128-row block instead of 8)

PSUM pools are partitioned so no cross-phase round-robin chain gates an
attention phase on a collective: scores 2x[128,1024], pat 2x[128,512],
misc (QKV/transpose/outproj) 2x[128,512] = 8 banks.

NOTE fp8 was tried and reverted: e4m3 probs/v/attn quantization is
deterministic per key, so errors correlate across the softmax sum instead
of averaging; measured rel err 2-7e-2 vs the 2e-2 gate (numpy-emulated
and CoreSim-confirmed, kernel_v2_299us.py keeps the DoubleRow variant).

Host side: pre-transposes hs and weights into PE layouts (bf16), runs the
SPMD NEFF on 8 cores via run_bass_kernel_spmd, reassembles [6,2048,320] f32
and adds bo. TimelineSim 330us (baseline 450us); rel err ~8.1e-3.
"""

import os
import sys

sys.path.insert(0, "/opt/trn_rl_repo")

import numpy as np

import concourse.bass as bass
import concourse.tile as tile
from concourse import bacc, mybir
from concourse.masks import make_identity

B, S, C, H, D = 6, 2048, 320, 8, 40
NCORES = 8
SSH = S // NCORES  # 256, sequence slice per core after AllToAll
EPS = 1e-5
SCALE = 1.0 / float(np.sqrt(D))
F32 = mybir.dt.float32
# dtype used for matmul operands (flip to float32r / bfloat16 for speed)
MM_DT = mybir.dt.bfloat16
# fp8 e4m3 for the PV contraction (DoubleRow perf mode, 2x MACs): probs are
# exp(scores) in [0.29, 3.4] and v in [-2.3, 2.3] -- well inside e4m3 range
F8 = mybir.dt.float8e4
# A2A payload dtype: fp8 halves collective + staging bytes; outproj averages
# the quantization over its C=320 contraction (flip to MM_DT to revert)
AT_DT = MM_DT


def fr(ap):
    return ap
KCH = [(0, 128), (128, 128), (256, 64)]  # contraction chunks of C=320


def build_nc(reps=1, collectives=True):
    nc = bacc.Bacc("TRN2", target_bir_lowering=False, debug=False,
                   num_devices=NCORES)

    hst = nc.dram_tensor("hst", [B, 384, S], MM_DT, kind="ExternalInput").ap()
    wqk = nc.dram_tensor("wqk", [C, 128], MM_DT, kind="ExternalInput").ap()
    wv = nc.dram_tensor("wv", [C, D], MM_DT, kind="ExternalInput").ap()
    wot = nc.dram_tensor("wot", [4, 2 * D, C], MM_DT, kind="ExternalInput").ap()
    out = nc.dram_tensor("out", [B, SSH, C], MM_DT, kind="ExternalOutput").ap()

    from contextlib import ExitStack

    with tile.TileContext(nc) as tc:
        with ExitStack() as ctx:
            singles = ctx.enter_context(tc.tile_pool(name="singles", bufs=1))
            hstp = ctx.enter_context(tc.tile_pool(name="hstp", bufs=4))
            qkp = ctx.enter_context(tc.tile_pool(name="qkp", bufs=2))
            k0p = ctx.enter_context(tc.tile_pool(name="k0p", bufs=2))
            vtp = ctx.enter_context(tc.tile_pool(name="vtp", bufs=2))
            vp = ctx.enter_context(tc.tile_pool(name="vp", bufs=2))
            tmpp = ctx.enter_context(tc.tile_pool(name="tmpp", bufs=2))
            probsp = ctx.enter_context(tc.tile_pool(name="probsp", bufs=6))
            attnp = ctx.enter_context(tc.tile_pool(name="attnp", bufs=2))
            recvp = ctx.enter_context(tc.tile_pool(name="recvp", bufs=2))
            outp = ctx.enter_context(tc.tile_pool(name="outp", bufs=2))
            statp = ctx.enter_context(tc.tile_pool(name="statp", bufs=8))
            rzp = ctx.enter_context(tc.tile_pool(name="rzp", bufs=2))
            bcp = ctx.enter_context(tc.tile_pool(name="bcp", bufs=2))
            ps_sc = ctx.enter_context(
                tc.tile_pool(name="ps_sc", bufs=2, space="PSUM"))
            ps_at = ctx.enter_context(
                tc.tile_pool(name="ps_at", bufs=2, space="PSUM"))
            ps_misc = ctx.enter_context(
                tc.tile_pool(name="ps_misc", bufs=2, space="PSUM"))
            dramp = ctx.enter_context(
                tc.tile_pool(name="dramp", bufs=1, space="DRAM"))

            # ---- constants ----
            ident = singles.tile([128, 128], MM_DT)
            make_identity(nc, ident)
            eps_t = singles.tile([128, 1], F32)
            nc.vector.memset(eps_t, EPS)

            wqk_sb = singles.tile([128, 3, 128], MM_DT)
            wv_sb = singles.tile([128, 3, D], MM_DT)
            wot_sb = singles.tile([2 * D, 4, C], MM_DT)
            for ck, (k0, kn) in enumerate(KCH):
                nc.sync.dma_start(wqk_sb[0:kn, ck, :], wqk[k0:k0 + kn, :])
                nc.sync.dma_start(wv_sb[0:kn, ck, :], wv[k0:k0 + kn, :])
            for j in range(4):
                nc.sync.dma_start(wot_sb[:, j, :], wot[j])

            # style stats saved from batch 1 (for 2) and 4 (for 5),
            # mirrored at partition bases 0 and 64 for both pair halves
            style = {n: singles.tile([128, 2], F32, tag=f"style_{n}",
                                     name=f"style_{n}")
                     for n in ("q", "k", "v")}

            def seq_stats(x, base):
                """mean and var' = var*S/(S-1) + eps over the free axis of
                x [D, S] (no ACT usage; keeps ACT exp-only)."""
                o = slice(base, base + D)
                st = statp.tile([128, 4, 6], F32, tag="bnst", name="st")
                for g in range(4):
                    nc.vector.bn_stats(st[o, g, :], x[:, g * 512:(g + 1) * 512])
                mv = statp.tile([128, 2], F32, tag="mv", name="mv")
                nc.vector.bn_aggr(mv[o, :], st[o])
                vp_ = statp.tile([128, 1], F32, tag="vp_", name="vp_")
                nc.vector.tensor_scalar(
                    out=vp_[o], in0=mv[o, 1:2],
                    scalar1=float(S) / float(S - 1), scalar2=EPS,
                    op0=mybir.AluOpType.mult, op1=mybir.AluOpType.add)
                return mv, vp_

            def adain(x, name, b, base):
                """AdaIN on x [D, S] at partition base 0/64: save style
                stats at b in {1,4}; restyle at b in {2,5}.
                a = sqrt(vs'/v') via r*rsqrt(r), rsqrt by 3 Newton steps
                from y0=1 (exact enough for var ratios near 1)."""
                if b in (0, 3):
                    return
                o = slice(base, base + D)
                mv, vp_ = seq_stats(x, base)
                if b in (1, 4):
                    for dst in (slice(0, D), slice(64, 64 + D)):
                        nc.vector.tensor_copy(style[name][dst, 0:1],
                                              mv[o, 0:1])
                        nc.vector.tensor_copy(style[name][dst, 1:2], vp_[o])
                else:
                    r = statp.tile([128, 1], F32, tag="r", name="r")
                    nc.vector.reciprocal(r[o], vp_[o])
                    nc.vector.tensor_mul(r[o], style[name][o, 1:2], r[o])
                    y = statp.tile([128, 1], F32, tag="y", name="y")
                    t = statp.tile([128, 1], F32, tag="t", name="t")
                    # y = (3 - r)/2  (first Newton step from y0=1)
                    nc.vector.tensor_scalar(
                        out=y[o], in0=r[o], scalar1=-0.5, scalar2=1.5,
                        op0=mybir.AluOpType.mult, op1=mybir.AluOpType.add)
                    for _ in range(2):
                        nc.vector.tensor_mul(t[o], y[o], y[o])
                        nc.vector.tensor_mul(t[o], t[o], r[o])
                        nc.vector.tensor_scalar(
                            out=t[o], in0=t[o], scalar1=-0.5, scalar2=1.5,
                            op0=mybir.AluOpType.mult, op1=mybir.AluOpType.add)
                        nc.vector.tensor_mul(y[o], y[o], t[o])
                    a = statp.tile([128, 1], F32, tag="a", name="a")
                    nc.vector.tensor_mul(a[o], r[o], y[o])
                    am = statp.tile([128, 1], F32, tag="am", name="am")
                    nc.vector.tensor_mul(am[o], mv[o, 0:1], a[o])
                    bvec = statp.tile([128, 1], F32, tag="bvec", name="bvec")
                    nc.vector.tensor_sub(bvec[o], style[name][o, 0:1], am[o])
                    nc.vector.tensor_scalar(
                        out=x, in0=x, scalar1=a[o], scalar2=bvec[o],
                        op0=mybir.AluOpType.mult, op1=mybir.AluOpType.add)

            for _rep in range(reps):
              sends, recvs = [], []
              for g, nb in ((0, 4), (1, 2)):
                  sends.append(dramp.tile([NCORES, nb, D, SSH], AT_DT,
                                          tag=f"sendg{g}", name=f"sendg{g}"))
                  recvs.append(dramp.tile([NCORES, nb, D, SSH], AT_DT,
                                          tag=f"recvg{g}", name=f"recvg{g}"))
              def prep_load(p):
                be, bo = 2 * p, 2 * p + 1
                # ---- load hs^T for both batches of the pair ----
                hst_e = hstp.tile([128, 3, S], MM_DT, tag="hst", name="hst_e")
                hst_o = hstp.tile([128, 3, S], MM_DT, tag="hst", name="hst_o")
                for hs_sb, bb in ((hst_e, be), (hst_o, bo)):
                    hsrc = hst[bb].rearrange("(ck p) s -> p ck s", p=128)
                    for hc in range(4):
                        cs = slice(hc * 512, (hc + 1) * 512)
                        nc.sync.dma_start(hs_sb[:, :, cs], hsrc[:, :, cs])

                # pair layout: even batch at partition base 0, odd at base 64
                qT2 = qkp.tile([128, S], MM_DT)
                kT2 = k0p.tile([128, S], MM_DT)
                vT2 = vtp.tile([128, S], MM_DT)
                tmp = tmpp.tile([128, S], MM_DT)
                return dict(p=p, hst_e=hst_e, hst_o=hst_o, qT2=qT2,
                            kT2=kT2, vT2=vT2, tmp=tmp)

              def prep_qkv(st, qc):
                    hst_e, hst_o = st["hst_e"], st["hst_o"]
                    qT2, kT2 = st["qT2"], st["kT2"]
                    vT2, tmp = st["vT2"], st["tmp"]
                    ns = slice(qc * 512, (qc + 1) * 512)
                    pqk = ps_misc.tile([128, 512], F32, tag="misc", name="pqk")
                    for ck, (k0, kn) in enumerate(KCH):
                        nc.tensor.matmul(
                            pqk, lhsT=fr(wqk_sb[0:kn, ck, :]),
                            rhs=fr(hst_e[0:kn, ck, ns]),
                            start=(ck == 0), stop=(ck == 2))
                    nc.vector.tensor_copy(qT2[0:D, ns], pqk[0:D, :])
                    nc.vector.tensor_copy(tmp[64:64 + D, ns], pqk[64:64 + D, :])
                    pqk2 = ps_misc.tile([128, 512], F32, tag="misc", name="pqk2")
                    for ck, (k0, kn) in enumerate(KCH):
                        nc.tensor.matmul(
                            pqk2, lhsT=fr(wqk_sb[0:kn, ck, :]),
                            rhs=fr(hst_o[0:kn, ck, ns]),
                            start=(ck == 0), stop=(ck == 2))
                    nc.vector.tensor_copy(kT2[64:64 + D, ns], pqk2[64:64 + D, :])
                    nc.vector.tensor_copy(tmp[0:D, ns], pqk2[0:D, :])
                    # v pair, col-packed: v_e -> psum rows 0:40, v_o -> 64:104
                    pv = ps_misc.tile([128, 512], F32, tag="misc", name="pv")
                    for ck, (k0, kn) in enumerate(KCH):
                        nc.tensor.matmul(
                            pv[0:D, :], lhsT=fr(wv_sb[0:kn, ck, :]),
                            rhs=fr(hst_e[0:kn, ck, ns]),
                            start=(ck == 0), stop=(ck == 2))
                    for ck, (k0, kn) in enumerate(KCH):
                        nc.tensor.matmul(
                            pv[64:64 + D, :], lhsT=fr(wv_sb[0:kn, ck, :]),
                            rhs=fr(hst_o[0:kn, ck, ns]),
                            start=(ck == 0), stop=(ck == 2))
                    nc.vector.tensor_copy(vT2[0:D, ns], pv[0:D, :])
                    nc.vector.tensor_copy(vT2[64:64 + D, ns], pv[64:64 + D, :])

              def prep_finish(st):
                p = st["p"]
                be, bo = 2 * p, 2 * p + 1
                qT2, kT2, vT2, tmp = st["qT2"], st["kT2"], st["vT2"], st["tmp"]
                # shifts: k_e staged at tmp[64:104] -> kT2 base 0;
                # q_o staged at tmp[0:40] -> qT2 base 64
                nc.sync.dma_start(kT2[0:D, :], tmp[64:64 + D, :])
                nc.sync.dma_start(qT2[64:64 + D, :], tmp[0:D, :])

                # ---- AdaIN (even at base 0, odd at base 64) ----
                adain(qT2[0:D, :], "q", be, 0)
                adain(kT2[0:D, :], "k", be, 0)
                adain(vT2[0:D, :], "v", be, 0)
                adain(qT2[64:64 + D, :], "q", bo, 64)
                adain(kT2[64:64 + D, :], "k", bo, 64)
                adain(vT2[64:64 + D, :], "v", bo, 64)

                # ---- v transposes (fp8, DoubleRow layout: kc pairs) ----
                v_sbs = {}
                for half, base in ((0, 0), (1, 64)):
                    v_sb = vp.tile([128, 16, 65], MM_DT, tag=f"v_sb{half}",
                                   name=f"v_sb{half}")
                    nc.vector.memset(v_sb[:, :, D:64], 0.0)
                    nc.vector.memset(v_sb[:, :, 64:65], 1.0)
                    v_sbs[half] = v_sb
                for sc in range(16):
                    for half, base in ((0, 0), (1, 64)):
                        tp = ps_misc.tile([128, 512], MM_DT, tag="misc",
                                          name="tp")
                        nc.tensor.transpose(
                            fr(tp[:, 0:D]),
                            fr(vT2[base:base + D, sc * 128:(sc + 1) * 128]),
                            fr(ident[base:base + D, base:base + D]))
                        nc.vector.tensor_copy(v_sbs[half][:, sc, 0:D],
                                              tp[:, 0:D])
                return qT2, kT2, v_sbs

              def attn_pair(p, qT2, kT2, v_sbs, post_qc=None):
                be, bo = 2 * p, 2 * p + 1
                # ---- attention: QK^T (bf16) -> exp (fp8 out) -> PV in fp8
                # DoubleRow (contracts kc pairs of 128 keys at 0.5 cyc/row)
                at_e = attnp.tile([D, S], AT_DT, tag="at_e", name="at_e")
                at_o = attnp.tile([D, S], AT_DT, tag="at_o", name="at_o")
                for qc in range(4):
                    qs = slice(qc * 512, (qc + 1) * 512)
                    # one PSUM bank each: v-out rows 0:40, Z row at aligned
                    # partition 64 (engine reads must start at 0/32/64/96)
                    pat_e = ps_at.tile([128, 512], F32, tag="attn",
                                       name="pat_e")
                    pat_o = ps_at.tile([128, 512], F32, tag="attn",
                                       name="pat_o")
                    def dr_pv(kc, pb):
                        for half, pat in ((0, pat_e), (1, pat_o)):
                            nc.tensor.matmul(
                                pat[0:65, :],
                                lhsT=fr(v_sbs[half][:, kc, :]),
                                rhs=fr(pb[:, half, :]),
                                start=(kc == 0), stop=(kc == 15))

                    # software-pipelined: DR of kc2-1 is emitted AFTER the
                    # QK+exp of kc2 so the in-order PE queue never parks on
                    # the ACT-gated DR ahead of the next chunk's QK
                    prev = None
                    for kc in range(16):
                        # pb: [part, batch, q-cols]
                        pb = probsp.tile([128, 2, 512], MM_DT, name="pb")
                        ks = slice(kc * 128, (kc + 1) * 128)
                        psc = ps_sc.tile([128, 1024], F32, tag="scores",
                                         name="psc")
                        nc.tensor.matmul(psc[:, 0:512],
                                         lhsT=fr(kT2[0:D, ks]),
                                         rhs=fr(qT2[0:D, qs]),
                                         start=True, stop=True)
                        nc.tensor.matmul(psc[:, 512:1024],
                                         lhsT=fr(kT2[64:64 + D, ks]),
                                         rhs=fr(qT2[64:64 + D, qs]),
                                         start=True, stop=True)
                        nc.scalar.activation(
                            pb, psc, mybir.ActivationFunctionType.Exp,
                            scale=SCALE)
                        if prev is not None:
                            dr_pv(*prev)
                        prev = (kc, pb)
                    dr_pv(*prev)
                    for pat, at in ((pat_e, at_e), (pat_o, at_o)):
                        rz = rzp.tile([1, 512], MM_DT, name="rz")
                        with nc.allow_low_precision(reason="1/Z in bf16"):
                            nc.vector.reciprocal(rz, pat[64:65, :])
                        # 1/Z broadcast on the (otherwise idle) gpsimd
                        # engine -- keeps PE and PSUM out of the normalize
                        bc = bcp.tile([D, 512], MM_DT, name="bc")
                        nc.gpsimd.partition_broadcast(bc, rz[0:1, :])
                        nc.vector.tensor_mul(at[:, qs], pat[0:D, :], bc)
                    if post_qc is not None:
                        post_qc(qc)

                # ---- stage sends into the grouped AllToAll buffer ----
                for b, attnT in ((be, at_e), (bo, at_o)):
                    g, bl = (0, b) if b < 4 else (1, b - 4)
                    nc.sync.dma_start(
                        sends[g][:, bl, :, :].rearrange("j d s -> d j s"),
                        attnT[:, :].rearrange("d (j s) -> d j s", j=NCORES))

              # ---- grouped AllToAll + output projections ----
              def outproj(b, recv_g, bl):
                  # recv head-blocks packed two heads deep (K=80) so the
                  # Wo contraction runs 4 matmuls instead of 8
                  ar = recvp.tile([2 * D, 4, SSH], AT_DT, name="ar")
                  rg2 = recv_g[:, bl, :, :].rearrange(
                      "(j two) d s -> two d j s", two=2)
                  nc.sync.dma_start(ar[0:D, :, :], rg2[0])
                  nc.sync.dma_start(ar[D:2 * D, :, :], rg2[1])
                  ob = outp.tile([128, 2, C], MM_DT, name="ob")
                  for m in range(2):
                      po = ps_misc.tile([128, 512], F32, tag="misc",
                                        name="po")
                      for j in range(4):
                          nc.tensor.matmul(
                              po[:, 0:C],
                              lhsT=fr(ar[:, j, m * 128:(m + 1) * 128]),
                              rhs=fr(wot_sb[:, j, :]),
                              start=(j == 0), stop=(j == 3))
                      nc.vector.tensor_copy(ob[:, m, :], po[:, 0:C])
                  nc.sync.dma_start(
                      out[b].rearrange("(m p) f -> p m f", p=128), ob)

              def a2a(g):
                  if collectives:
                      nc.gpsimd.collective_compute(
                          "AllToAll", mybir.AluOpType.bypass,
                          replica_groups=[list(range(NCORES))],
                          ins=[sends[g][:, :, :, :].opt()],
                          outs=[recvs[g][:, :, :, :].opt()])

              # schedule: pair-2 prep is emitted BEFORE a2a0+outproj so its
              # ps_misc tiles are not chained behind outproj's (which gate
              # on the collective) -- keeps the ACT-bound attention phases
              # back-to-back across pairs
              # pair p+1's QKV chunks are interleaved into pair p's
              # (ACT-bound) attention via post_qc so the in-order PE queue
              # never drains attention before starting the next projections
              st0 = prep_load(0)
              for qc in range(4):
                  prep_qkv(st0, qc)
              s0 = prep_finish(st0)
              st1 = prep_load(1)
              attn_pair(0, *s0, post_qc=lambda qc: prep_qkv(st1, qc))
              s1 = prep_finish(st1)
              st2 = prep_load(2)
              attn_pair(1, *s1, post_qc=lambda qc: prep_qkv(st2, qc))
              s2 = prep_finish(st2)
              a2a(0)

              # group-0 outproj interleaved late (qc 2-3) in pair-2's
              # attention: PE picks the (a2a0-gated) po matmuls up during
              # the ACT-bound inner loop instead of stalling ahead of it
              def g0_outproj(qc):
                  if qc in (2, 3):
                      for b in (2 * qc - 4, 2 * qc - 3):
                          outproj(b, recvs[0], b)

              attn_pair(2, *s2, post_qc=g0_outproj)
              a2a(1)
              for b in range(4, 6):
                  outproj(b, recvs[1], b - 4)

    nc.compile()
    return nc


_NC_CACHE = {}


def _get_nc(reps=1):
    key = f"nc{reps}"
    if key not in _NC_CACHE:
        _NC_CACHE[key] = build_nc(reps)
    return _NC_CACHE[key]


import ml_dtypes


def _np_mm(x):
    return np.ascontiguousarray(np.asarray(x).astype(ml_dtypes.bfloat16))


def make_in_maps(hidden_states, Wq, Wk, Wv, Wo):
    hsT = np.zeros((B, 384, S), np.float32)
    hsT[:, 0:C, :] = np.transpose(np.asarray(hidden_states, np.float32),
                                  (0, 2, 1))
    hsT = _np_mm(hsT)  # [B, 384, S] bf16, rows 320:384 zero
    # Wo.T channel rows grouped as head pairs: [4, 80, C]
    woT = _np_mm(np.ascontiguousarray(Wo.T).reshape(4, 2 * D, C))
    in_maps = []
    for h in range(NCORES):
        sl = slice(h * D, (h + 1) * D)
        stack = np.zeros((128, C), np.float32)
        stack[0:D] = Wq[sl]
        stack[64:64 + D] = Wk[sl]
        in_maps.append({
            "hst": hsT,
            "wqk": _np_mm(stack.T),
            "wv": _np_mm(Wv[sl].T),
            "wot": woT,
        })
    return in_maps


def assemble(results, bo):
    full = np.empty((B, S, C), np.float32)
    for i in range(NCORES):
        full[:, i * SSH:(i + 1) * SSH, :] = results[i]["out"]
    full += np.asarray(bo, np.float32)[None, None, :]
    return full


def run_hw(inputs, trace=False):
    from concourse.bass_utils import run_bass_kernel_spmd
    nc = _get_nc()
    in_maps = make_in_maps(inputs["hidden_states"], inputs["Wq"],
                           inputs["Wk"], inputs["Wv"], inputs["Wo"])
    res = run_bass_kernel_spmd(nc, in_maps, core_ids=list(range(NCORES)),
                               trace=trace)
    return assemble(res.results, inputs["bo"]), res


def kernel(**inputs):
    out, _ = run_hw(inputs, trace=False)
    return out

